# revision 22
# baseline (speedup 1.0000x reference)
"""Sliding-window causal self-attention (B=2, T=2048, C=1024, H=16, Dh=64,
window=256) + QKV/out projections, sharded over 8 NeuronCores as
data-parallel over B (2) x tensor-parallel over head groups (4 heads/core).

Layout strategy: activations kept "token-transposed" (features on
partitions, tokens free) for the projections.  Attention scores are
computed TRANSPOSED (sT[k, q] = matmul(lhsT=k_tile, rhs=q_tile)) so that
exp(sT) is directly the P^T the PV matmul needs -- no PE transposes and no
diag-normalization matmuls.  Row sums come for free from a ones-column
appended to V inside the PV matmul; normalization is a reciprocal +
DMA-broadcast + one fused multiply into the attnT staging tile.
Band masking is applied post-exp with affine_select on the (idle) GpSimd
engine.  Input DMAs are chunked and ordered so the QKV matmuls start as
soon as the first weight/token chunks land, with PE-warmup matmuls
covering the HAM cold window during the DMA prologue.
"""

import math

import numpy as np

B = 2
T = 2048
C = 1024
H = 16
DH = 64
WINDOW = 256
HEADS_PER_CORE = 4
N_CORES = 8
QT = T // 128  # 16 query tiles of 128
FQ = HEADS_PER_CORE * DH  # 256 local features
VG = DH + 1  # v-block column group: 64 v columns + 1 ones column

_PROGRAM = None  # compile once per process


def _emit(nc, tc, aps, ctx):
    from concourse import mybir
    from concourse.bass import AP

    f32 = mybir.dt.float32
    bf16 = mybir.dt.bfloat16
    Exp = mybir.ActivationFunctionType.Exp
    Copy = mybir.ActivationFunctionType.Copy
    GE = mybir.AluOpType.is_ge

    xT, wT, woT, cos4, sin4, y = (
        aps["xT"], aps["wT"], aps["woT"], aps["cos4"], aps["sin4"], aps["y"],
    )

    consts = ctx.enter_context(tc.tile_pool(name="consts", bufs=1))
    stage = ctx.enter_context(tc.tile_pool(name="stage", bufs=1))
    work = ctx.enter_context(tc.tile_pool(name="work", bufs=3))
    tmp = ctx.enter_context(tc.tile_pool(name="tmp", bufs=4))
    small = ctx.enter_context(tc.tile_pool(name="small", bufs=4))
    ysbp = ctx.enter_context(tc.tile_pool(name="ysbp", bufs=2))
    pA = ctx.enter_context(tc.tile_pool(name="pA", bufs=2, space="PSUM"))
    pB = ctx.enter_context(tc.tile_pool(name="pB", bufs=2, space="PSUM"))
    pP = ctx.enter_context(tc.tile_pool(name="pP", bufs=1, space="PSUM"))
    pO = ctx.enter_context(tc.tile_pool(name="pO", bufs=3, space="PSUM"))
    p1 = [pA, pB, pP, pO]

    # ---- SBUF residents ----
    xT_sb = consts.tile([128, 8 * T], bf16, tag="xT")  # [C-part, (kc t)]
    wT_sb = consts.tile([128, 8 * 768], bf16, tag="wT")
    woT_sb = consts.tile([128, 2 * C], bf16, tag="woT")
    cos_sb = consts.tile([128, T], bf16, tag="cos")
    sin_sb = consts.tile([128, T], bf16, tag="sin")
    warm_sb = consts.tile([128, 640], bf16, tag="warm")

    rot = [stage.tile([128, T], bf16, tag=f"rot{i}", name=f"rot{i}")
           for i in range(4)]
    qhT = stage.tile([64, HEADS_PER_CORE * T], bf16, tag="qhT")
    khT = stage.tile([64, HEADS_PER_CORE * T], bf16, tag="khT")
    # v blocks: per key tile kt, per head: 64 v columns + a ones column
    v4 = stage.tile([128, QT * 4 * VG], bf16, tag="v4")
    attnT = stage.tile([128, 2 * T], bf16, tag="attnT")  # [(f%128), (kc t)]

    # ---- warmup / init (no DMA deps) ----
    nc.vector.memset(warm_sb, 0.0)
    v4v = v4.rearrange("p (kt h g) -> p kt h g", kt=QT, h=4)
    nc.gpsimd.memset(v4v[:, :, :, DH:DH + 1], 1.0)
    zero_reg = nc.gpsimd.to_reg(0.0)
    # touch every gpsimd ucode op once now: the library reload that pulls in
    # PartitionBroadcast/affine_select costs ~7us and must not hit mid-loop
    warm_g = stage.tile([64, 512], f32, tag="warmg")
    nc.gpsimd.memset(warm_g[0:1, :], 1.0)
    nc.gpsimd.partition_broadcast(warm_g, warm_g[0:1, :])
    warm_h = stage.tile([128, 128], bf16, tag="warmh")
    nc.gpsimd.memset(warm_h, 0.0)
    nc.gpsimd.affine_select(warm_h, warm_h, pattern=[[1, 128]],
                            compare_op=GE, fill=zero_reg, base=0,
                            channel_multiplier=-1)
    for i in range(36):  # keep the PE HAM-warm while input DMAs stream in
        acc = p1[i % 4].tile([128, 512], f32, tag="p")
        nc.tensor.matmul(acc, lhsT=warm_sb[:, :128], rhs=warm_sb[:, 128:640],
                         start=True, stop=True)

    # ---- input DMAs, in consumption order ----
    wT_v = wT_sb.rearrange("p (kc f) -> p kc f", kc=8)
    wT_in = wT.rearrange("p (kc f) -> p kc f", kc=8)
    nc.sync.dma_start(out=wT_v[:, :, 0:512], in_=wT_in[:, :, 0:512])
    xT_v = xT_sb.rearrange("p (kc t) -> p kc t", kc=8)
    xT_in = xT.rearrange("p (kc t) -> p kc t", kc=8)
    for s in range(4):
        tsl = slice(s * 512, (s + 1) * 512)
        nc.sync.dma_start(out=xT_v[:, :, tsl], in_=xT_in[:, :, tsl])
        nc.sync.dma_start(out=cos_sb[:, tsl], in_=cos4[:, tsl])
        nc.sync.dma_start(out=sin_sb[:, tsl], in_=sin4[:, tsl])
    nc.sync.dma_start(out=wT_v[:, :, 512:768], in_=wT_in[:, :, 512:768])
    nc.sync.dma_start(out=woT_sb, in_=woT)

    # ---- phase 1: q/k projection + RoPE (reads PSUM directly) ----
    for s in range(4):  # 512-token slices
        tsl = slice(s * 512, (s + 1) * 512)
        accs = []
        for blk in range(4):  # q_x1 q_x2 k_x1 k_x2
            acc = p1[blk].tile([128, 512], f32, tag="p")
            for kc in range(8):
                nc.tensor.matmul(
                    acc,
                    lhsT=wT_sb[:, kc * 768 + blk * 128:kc * 768 + (blk + 1) * 128],
                    rhs=xT_sb[:, kc * T + s * 512:kc * T + (s + 1) * 512],
                    start=(kc == 0),
                    stop=(kc == 7),
                )
            accs.append(acc)
        # RoPE: rot1 = x1*cos - x2*sin ; rot2 = x2*cos + x1*sin
        for pair in range(2):  # 0 -> q, 1 -> k
            x1, x2 = accs[2 * pair], accs[2 * pair + 1]
            r1, r2 = rot[2 * pair][:, tsl], rot[2 * pair + 1][:, tsl]
            t1 = tmp.tile([128, 512], bf16, tag="t1")
            t2 = tmp.tile([128, 512], bf16, tag="t2")
            t3 = tmp.tile([128, 512], bf16, tag="t3")
            t4 = tmp.tile([128, 512], bf16, tag="t4")
            nc.vector.tensor_mul(t1, x1, cos_sb[:, tsl])
            nc.vector.tensor_mul(t2, x2, sin_sb[:, tsl])
            nc.gpsimd.tensor_sub(r1, t1, t2)
            nc.vector.tensor_mul(t3, x2, cos_sb[:, tsl])
            nc.vector.tensor_mul(t4, x1, sin_sb[:, tsl])
            nc.gpsimd.tensor_add(r2, t3, t4)
        # repack this token slice into head-contiguous [64, (h t)] layout
        for hl in range(HEADS_PER_CORE):
            hsl = slice(hl * T + s * 512, hl * T + (s + 1) * 512)
            for half in range(2):
                psl = slice(half * 32, (half + 1) * 32)
                rsl = slice(hl * 32, (hl + 1) * 32)
                nc.sync.dma_start(out=qhT[psl, hsl], in_=rot[half][rsl, tsl])
                nc.sync.dma_start(out=khT[psl, hsl], in_=rot[2 + half][rsl, tsl])

    # ---- phase 2: software-pipelined attention ----
    # per iteration i: v(i), bcast(i-1), attention(i)+recip(i),
    # normalize-muls(i-1), out-proj(i-1).  The tensor queue never waits on
    # the reciprocal: by the time it reaches bcast(i-1), rc4(i-1) is old.
    o4s = [None] * QT
    rc4s = [None] * QT
    rcbs = [None] * QT

    def bcast(qt):  # rcb[d, (h q)] = 1/rowsum broadcast over partitions
        # gpsimd, emitted after a head-loop's masks: it executes in the
        # mask-free window while PV3/outproj run, never blocking a mask
        rcbs[qt] = work.tile([64, 512], f32, tag="rcb", name=f"rcb{qt}")
        nc.gpsimd.partition_broadcast(rcbs[qt], rc4s[qt])

    def norm_muls(qt):  # attnT <- o4 * rcb  (both PSUM)
        ov = o4s[qt][0:64, :].rearrange("p (h q) -> p h q", h=4)
        rv = rcbs[qt].rearrange("p (h q) -> p h q", h=4)
        av = attnT.rearrange("p (b t) -> p b t", b=2)
        qsl = slice(qt * 128, (qt + 1) * 128)
        nc.vector.tensor_mul(av[0:64, :, qsl], ov[:, 0::2, :], rv[:, 0::2, :])
        nc.vector.tensor_mul(av[64:128, :, qsl], ov[:, 1::2, :], rv[:, 1::2, :])

    def outproj(qt):
        ysb = ysbp.tile([128, C], bf16, tag="ysb")
        for nh in range(2):
            acc = (pA if nh == 0 else pB).tile([128, 512], f32, tag="p")
            for kc in range(2):
                nc.tensor.matmul(
                    acc,
                    lhsT=attnT[:, kc * T + qt * 128:kc * T + (qt + 1) * 128],
                    rhs=woT_sb[:, kc * C + nh * 512:kc * C + (nh + 1) * 512],
                    start=(kc == 0),
                    stop=(kc == 1),
                )
            if nh == 0:  # vector is idle mid-iteration; scalar still on exps
                nc.vector.tensor_copy(ysb[:, nh * 512:(nh + 1) * 512], acc)
            else:
                nc.scalar.copy(ysb[:, nh * 512:(nh + 1) * 512], acc)
        nc.sync.dma_start(out=y[qt * 128:(qt + 1) * 128, :], in_=ysb)

    for qt in range(QT):
        # v tile qt in natural (token-partition) layout, strided into v4
        acc = pA.tile([128, FQ], f32, tag="p")
        for kc in range(8):
            nc.tensor.matmul(
                acc,
                lhsT=xT_sb[:, kc * T + qt * 128:kc * T + (qt + 1) * 128],
                rhs=wT_sb[:, kc * 768 + 512:kc * 768 + 768],
                start=(kc == 0),
                stop=(kc == 7),
            )
        nc.any.tensor_copy(
            v4v[:, qt, :, 0:DH],
            acc.rearrange("p (h f) -> p h f", h=4),
        )

        nkt = min(qt + 1, 3)  # key tiles in window
        w = 128 * nkt
        kt0 = max(qt - 2, 0)
        o4 = o4s[qt] = pO.tile([65, 512], f32, tag="p", name=f"o4_{qt}")

        def pv(hl, pT):
            for a in range(nkt):
                kt = kt0 + a
                nc.tensor.matmul(
                    o4[:, hl * 128:(hl + 1) * 128],
                    lhsT=v4[:, (kt * 4 + hl) * VG:(kt * 4 + hl + 1) * VG],
                    rhs=pT[:, a * 128:(a + 1) * 128],
                    start=(a == 0),
                    stop=(a == nkt - 1),
                )

        pend = None  # (hl, pT) whose PV is issued under the NEXT head
        for hl in range(HEADS_PER_CORE):
            # transposed scores sT[k, q] accumulate straight into one bank
            sT = pB.tile([128, 384], f32, tag="p")
            for a in range(nkt):
                nc.tensor.matmul(
                    sT[:, a * 128:(a + 1) * 128],
                    lhsT=khT[:, hl * T + (kt0 + a) * 128:hl * T + (kt0 + a + 1) * 128],
                    rhs=qhT[:, hl * T + qt * 128:hl * T + (qt + 1) * 128],
                    start=True,
                    stop=True,
                )
            pT = work.tile([128, 384], bf16, tag="pT")
            nc.scalar.activation(pT[:, :w], sT[:, :w], Exp)
            # band masks, post-exp on GpSimd: diagonal block keeps k<=q,
            # far block (qt-2) keeps k>q; middle block is fully in-window
            d0 = (nkt - 1) * 128
            nc.gpsimd.affine_select(
                pT[:, d0:d0 + 128], pT[:, d0:d0 + 128], pattern=[[1, 128]],
                compare_op=GE, fill=zero_reg, base=0, channel_multiplier=-1)
            if qt >= 2:
                nc.gpsimd.affine_select(
                    pT[:, 0:128], pT[:, 0:128], pattern=[[-1, 128]],
                    compare_op=GE, fill=zero_reg, base=-1, channel_multiplier=1)
            if pend is not None:
                pv(*pend)
            pend = (hl, pT)
        if qt >= 1:
            bcast(qt - 1)
        if qt >= 2:
            norm_muls(qt - 2)
            outproj(qt - 2)  # tensor work covering the last head's exp+mask
        pv(*pend)
        rs_sb = small.tile([1, 512], f32, tag="rs")
        nc.scalar.activation(rs_sb, o4[64:65, :], Copy)
        rc4s[qt] = small.tile([1, 512], f32, tag="rc4", name=f"rc4_{qt}")
        nc.vector.reciprocal_approx_fast(rc4s[qt], rs_sb)
    norm_muls(QT - 2)
    outproj(QT - 2)
    bcast(QT - 1)
    norm_muls(QT - 1)
    outproj(QT - 1)


def _build_program():
    import concourse.tile as tile
    from concourse import bacc, mybir

    bf16 = mybir.dt.bfloat16

    nc = bacc.Bacc("TRN2", target_bir_lowering=False, debug=False,
                   num_devices=N_CORES)
    aps = {
        "xT": nc.dram_tensor("xT", [128, 8 * T], bf16, kind="ExternalInput").ap(),
        "wT": nc.dram_tensor("wT", [128, 8 * 768], bf16, kind="ExternalInput").ap(),
        "woT": nc.dram_tensor("woT", [128, 2 * C], bf16, kind="ExternalInput").ap(),
        "cos4": nc.dram_tensor("cos4", [128, T], bf16, kind="ExternalInput").ap(),
        "sin4": nc.dram_tensor("sin4", [128, T], bf16, kind="ExternalInput").ap(),
        "y": nc.dram_tensor("y", [T, C], bf16, kind="ExternalOutput").ap(),
    }
    from contextlib import ExitStack

    with tile.TileContext(nc) as tc, ExitStack() as ctx:
        _emit(nc, tc, aps, ctx)
    nc.compile()
    return nc


def _get_program():
    global _PROGRAM
    if _PROGRAM is None:
        _PROGRAM = _build_program()
    return _PROGRAM


def _host_inputs(x, w_qkv, w_out):
    import ml_dtypes

    bf16 = ml_dtypes.bfloat16
    x = np.asarray(x, np.float32)
    w_qkv = np.asarray(w_qkv, np.float32)
    w_out = np.asarray(w_out, np.float32)

    wq, wk, wv = w_qkv[0:C], w_qkv[C:2 * C], w_qkv[2 * C:3 * C]
    scale = 1.0 / math.sqrt(DH)

    # RoPE tables (transposed, tiled over the 4 heads of a block)
    inv_freq = 1.0 / (10000.0 ** (np.arange(0, DH, 2, dtype=np.float32) / DH))
    freqs = np.outer(np.arange(T, dtype=np.float32), inv_freq)  # [T, 32]
    cos4 = np.ascontiguousarray(np.tile(np.cos(freqs).T, (4, 1))).astype(bf16)
    sin4 = np.ascontiguousarray(np.tile(np.sin(freqs).T, (4, 1))).astype(bf16)

    def ilv(m):  # [(kc*128), F] -> [128, kc*F] kc-major interleave
        kc = m.shape[0] // 128
        return np.ascontiguousarray(
            m.reshape(kc, 128, -1).transpose(1, 0, 2).reshape(128, -1))

    xT = [ilv(x[b].T).astype(bf16) for b in range(B)]

    in_maps = []
    for core in range(N_CORES):
        b, g = divmod(core, 4)
        hs = range(4 * g, 4 * g + 4)
        rows = []
        for half in range(2):  # q_x1, q_x2
            rows.append(np.concatenate(
                [wq[h * DH + 32 * half:h * DH + 32 * half + 32] for h in hs]) * scale)
        for half in range(2):  # k_x1, k_x2
            rows.append(np.concatenate(
                [wk[h * DH + 32 * half:h * DH + 32 * half + 32] for h in hs]))
        rows.append(wv[g * FQ:(g + 1) * FQ])
        wmat = np.concatenate(rows)  # [768, C]
        wT = ilv(wmat.T).astype(bf16)
        woT = ilv(w_out[:, g * FQ:(g + 1) * FQ].T).astype(bf16)
        in_maps.append({
            "xT": xT[b], "wT": wT, "woT": woT, "cos4": cos4, "sin4": sin4,
        })
    return in_maps


def kernel(x, w_qkv, w_out, _trace=False):
    from concourse import bass_utils

    nc = _get_program()
    in_maps = _host_inputs(x, w_qkv, w_out)
    res = bass_utils.run_bass_kernel_spmd(
        nc, in_maps, core_ids=list(range(N_CORES)), trace=_trace,
    )
    parts = [res.results[core]["y"].astype(np.float32) for core in range(N_CORES)]
    out = np.stack([
        parts[0] + parts[1] + parts[2] + parts[3],
        parts[4] + parts[5] + parts[6] + parts[7],
    ])
    if _trace:
        return out, res
    return out


# revision 23
# speedup vs baseline: 1.2149x; 1.2149x over previous
"""Sliding-window causal self-attention (B=2, T=2048, C=1024, H=16, Dh=64,
window=256) + QKV/out projections, sharded over 8 NeuronCores as
data-parallel over B (2) x tensor-parallel over head groups (4 heads/core).

Layout strategy: activations kept "token-transposed" (features on
partitions, tokens free) for the projections.  Attention scores are
computed TRANSPOSED (sT[k, q] = matmul(lhsT=k_tile, rhs=q_tile)) so that
exp(sT) is directly the P^T the PV matmul needs -- no PE transposes and no
diag-normalization matmuls.  Row sums come for free from a ones-column
appended to V inside the PV matmul; normalization is a reciprocal +
DMA-broadcast + one fused multiply into the attnT staging tile.
Band masking is applied post-exp with affine_select on the (idle) GpSimd
engine.  Input DMAs are chunked and ordered so the QKV matmuls start as
soon as the first weight/token chunks land, with PE-warmup matmuls
covering the HAM cold window during the DMA prologue.
"""

import math

import numpy as np

B = 2
T = 2048
C = 1024
H = 16
DH = 64
WINDOW = 256
HEADS_PER_CORE = 4
N_CORES = 8
QT = T // 128  # 16 query tiles of 128
FQ = HEADS_PER_CORE * DH  # 256 local features
VG = DH + 1  # v-block column group: 64 v columns + 1 ones column

_PROGRAM = None  # compile once per process


def _emit(nc, tc, aps, ctx):
    from concourse import mybir
    from concourse.bass import AP

    f32 = mybir.dt.float32
    bf16 = mybir.dt.bfloat16
    Exp = mybir.ActivationFunctionType.Exp
    Copy = mybir.ActivationFunctionType.Copy
    GE = mybir.AluOpType.is_ge

    xT, wT, woT, cos4, sin4, y = (
        aps["xT"], aps["wT"], aps["woT"], aps["cos4"], aps["sin4"], aps["y"],
    )

    consts = ctx.enter_context(tc.tile_pool(name="consts", bufs=1))
    stage = ctx.enter_context(tc.tile_pool(name="stage", bufs=1))
    work = ctx.enter_context(tc.tile_pool(name="work", bufs=3))
    tmp = ctx.enter_context(tc.tile_pool(name="tmp", bufs=4))
    small = ctx.enter_context(tc.tile_pool(name="small", bufs=4))
    ysbp = ctx.enter_context(tc.tile_pool(name="ysbp", bufs=2))
    pA = ctx.enter_context(tc.tile_pool(name="pA", bufs=2, space="PSUM"))
    pB = ctx.enter_context(tc.tile_pool(name="pB", bufs=2, space="PSUM"))
    pP = ctx.enter_context(tc.tile_pool(name="pP", bufs=1, space="PSUM"))
    pO = ctx.enter_context(tc.tile_pool(name="pO", bufs=3, space="PSUM"))
    p1 = [pA, pB, pP, pO]

    # ---- SBUF residents ----
    xT_sb = consts.tile([128, 8 * T], bf16, tag="xT")  # [C-part, (kc t)]
    wT_sb = consts.tile([128, 8 * 768], bf16, tag="wT")
    woT_sb = consts.tile([128, 2 * C], bf16, tag="woT")
    cos_sb = consts.tile([128, T], bf16, tag="cos")
    sin_sb = consts.tile([128, T], bf16, tag="sin")
    warm_sb = consts.tile([128, 640], bf16, tag="warm")

    rot = [stage.tile([128, T], bf16, tag=f"rot{i}", name=f"rot{i}")
           for i in range(4)]
    qhT = stage.tile([64, HEADS_PER_CORE * T], bf16, tag="qhT")
    khT = stage.tile([64, HEADS_PER_CORE * T], bf16, tag="khT")
    # v blocks: per key tile kt, per head: 64 v columns + a ones column
    v4 = stage.tile([128, QT * 4 * VG], bf16, tag="v4")
    attnT = stage.tile([128, 2 * T], bf16, tag="attnT")  # [(f%128), (kc t)]

    # ---- warmup / init (no DMA deps) ----
    nc.vector.memset(warm_sb, 0.0)
    v4v = v4.rearrange("p (kt h g) -> p kt h g", kt=QT, h=4)
    nc.gpsimd.memset(v4v[:, :, :, DH:DH + 1], 1.0)
    zero_reg = nc.gpsimd.to_reg(0.0)
    # touch every gpsimd ucode op once now: the library reload that pulls in
    # PartitionBroadcast/affine_select costs ~7us and must not hit mid-loop
    warm_g = stage.tile([64, 512], f32, tag="warmg")
    nc.gpsimd.memset(warm_g[0:1, :], 1.0)
    nc.gpsimd.partition_broadcast(warm_g, warm_g[0:1, :])
    warm_h = stage.tile([128, 128], bf16, tag="warmh")
    nc.gpsimd.memset(warm_h, 0.0)
    nc.gpsimd.affine_select(warm_h, warm_h, pattern=[[1, 128]],
                            compare_op=GE, fill=zero_reg, base=0,
                            channel_multiplier=-1)
    for i in range(36):  # keep the PE HAM-warm while input DMAs stream in
        acc = p1[i % 4].tile([128, 512], f32, tag="p")
        nc.tensor.matmul(acc, lhsT=warm_sb[:, :128], rhs=warm_sb[:, 128:640],
                         start=True, stop=True)

    # ---- input DMAs, in consumption order ----
    wT_v = wT_sb.rearrange("p (kc f) -> p kc f", kc=8)
    wT_in = wT.rearrange("p (kc f) -> p kc f", kc=8)
    nc.sync.dma_start(out=wT_v[:, :, 0:512], in_=wT_in[:, :, 0:512])
    xT_v = xT_sb.rearrange("p (kc t) -> p kc t", kc=8)
    xT_in = xT.rearrange("p (kc t) -> p kc t", kc=8)
    for s in range(4):
        tsl = slice(s * 512, (s + 1) * 512)
        nc.sync.dma_start(out=xT_v[:, :, tsl], in_=xT_in[:, :, tsl])
        nc.sync.dma_start(out=cos_sb[:, tsl], in_=cos4[:, tsl])
        nc.sync.dma_start(out=sin_sb[:, tsl], in_=sin4[:, tsl])
    nc.sync.dma_start(out=wT_v[:, :, 512:768], in_=wT_in[:, :, 512:768])
    nc.sync.dma_start(out=woT_sb, in_=woT)

    # ---- phase 1: q/k projection + RoPE (reads PSUM directly) ----
    for s in range(4):  # 512-token slices
        tsl = slice(s * 512, (s + 1) * 512)
        accs = []
        for blk in range(4):  # q_x1 q_x2 k_x1 k_x2
            acc = p1[blk].tile([128, 512], f32, tag="p")
            for kc in range(8):
                nc.tensor.matmul(
                    acc,
                    lhsT=wT_sb[:, kc * 768 + blk * 128:kc * 768 + (blk + 1) * 128],
                    rhs=xT_sb[:, kc * T + s * 512:kc * T + (s + 1) * 512],
                    start=(kc == 0),
                    stop=(kc == 7),
                )
            accs.append(acc)
        # RoPE: rot1 = x1*cos - x2*sin ; rot2 = x2*cos + x1*sin
        for pair in range(2):  # 0 -> q, 1 -> k
            x1, x2 = accs[2 * pair], accs[2 * pair + 1]
            r1, r2 = rot[2 * pair][:, tsl], rot[2 * pair + 1][:, tsl]
            t1 = tmp.tile([128, 512], bf16, tag="t1")
            t2 = tmp.tile([128, 512], bf16, tag="t2")
            t3 = tmp.tile([128, 512], bf16, tag="t3")
            t4 = tmp.tile([128, 512], bf16, tag="t4")
            nc.vector.tensor_mul(t1, x1, cos_sb[:, tsl])
            nc.vector.tensor_mul(t2, x2, sin_sb[:, tsl])
            nc.vector.tensor_sub(r1, t1, t2)
            nc.vector.tensor_mul(t3, x2, cos_sb[:, tsl])
            nc.vector.tensor_mul(t4, x1, sin_sb[:, tsl])
            nc.vector.tensor_add(r2, t3, t4)
        # repack this token slice into head-contiguous [64, (h t)] layout
        for hl in range(HEADS_PER_CORE):
            hsl = slice(hl * T + s * 512, hl * T + (s + 1) * 512)
            for half in range(2):
                psl = slice(half * 32, (half + 1) * 32)
                rsl = slice(hl * 32, (hl + 1) * 32)
                nc.sync.dma_start(out=qhT[psl, hsl], in_=rot[half][rsl, tsl])
                nc.sync.dma_start(out=khT[psl, hsl], in_=rot[2 + half][rsl, tsl])

    # ---- phase 2: software-pipelined attention ----
    # per iteration i: v(i), bcast(i-1), attention(i)+recip(i),
    # normalize-muls(i-1), out-proj(i-1).  The tensor queue never waits on
    # the reciprocal: by the time it reaches bcast(i-1), rc4(i-1) is old.
    o4s = [None] * QT
    rc4s = [None] * QT
    rcbs = [None] * QT

    def bcast(qt):  # rcb[d, (h q)] = 1/rowsum broadcast over partitions
        # gpsimd, emitted after a head-loop's masks: it executes in the
        # mask-free window while PV3/outproj run, never blocking a mask
        rcbs[qt] = work.tile([64, 512], f32, tag="rcb", name=f"rcb{qt}")
        nc.gpsimd.partition_broadcast(rcbs[qt], rc4s[qt])

    def norm_muls(qt):  # attnT <- o4 * rcb  (both PSUM)
        ov = o4s[qt][0:64, :].rearrange("p (h q) -> p h q", h=4)
        rv = rcbs[qt].rearrange("p (h q) -> p h q", h=4)
        av = attnT.rearrange("p (b t) -> p b t", b=2)
        qsl = slice(qt * 128, (qt + 1) * 128)
        nc.vector.tensor_mul(av[0:64, :, qsl], ov[:, 0::2, :], rv[:, 0::2, :])
        nc.vector.tensor_mul(av[64:128, :, qsl], ov[:, 1::2, :], rv[:, 1::2, :])

    def outproj(qt):
        ysb = ysbp.tile([128, C], bf16, tag="ysb")
        for nh in range(2):
            acc = (pA if nh == 0 else pB).tile([128, 512], f32, tag="p")
            for kc in range(2):
                nc.tensor.matmul(
                    acc,
                    lhsT=attnT[:, kc * T + qt * 128:kc * T + (qt + 1) * 128],
                    rhs=woT_sb[:, kc * C + nh * 512:kc * C + (nh + 1) * 512],
                    start=(kc == 0),
                    stop=(kc == 1),
                )
            if nh == 0:  # vector is idle mid-iteration; scalar still on exps
                nc.vector.tensor_copy(ysb[:, nh * 512:(nh + 1) * 512], acc)
            else:
                nc.scalar.copy(ysb[:, nh * 512:(nh + 1) * 512], acc)
        nc.sync.dma_start(out=y[qt * 128:(qt + 1) * 128, :], in_=ysb)

    for qt in range(QT):
        # v tile qt in natural (token-partition) layout, strided into v4
        acc = pA.tile([128, FQ], f32, tag="p")
        for kc in range(8):
            nc.tensor.matmul(
                acc,
                lhsT=xT_sb[:, kc * T + qt * 128:kc * T + (qt + 1) * 128],
                rhs=wT_sb[:, kc * 768 + 512:kc * 768 + 768],
                start=(kc == 0),
                stop=(kc == 7),
            )
        nc.any.tensor_copy(
            v4v[:, qt, :, 0:DH],
            acc.rearrange("p (h f) -> p h f", h=4),
        )

        nkt = min(qt + 1, 3)  # key tiles in window
        w = 128 * nkt
        kt0 = max(qt - 2, 0)
        o4 = o4s[qt] = pO.tile([65, 512], f32, tag="p", name=f"o4_{qt}")

        def pv(hl, pT):
            for a in range(nkt):
                kt = kt0 + a
                nc.tensor.matmul(
                    o4[:, hl * 128:(hl + 1) * 128],
                    lhsT=v4[:, (kt * 4 + hl) * VG:(kt * 4 + hl + 1) * VG],
                    rhs=pT[:, a * 128:(a + 1) * 128],
                    start=(a == 0),
                    stop=(a == nkt - 1),
                )

        pend = None  # (hl, pT) whose PV is issued under the NEXT head
        for hl in range(HEADS_PER_CORE):
            # transposed scores sT[k, q] accumulate straight into one bank
            sT = pB.tile([128, 384], f32, tag="p")
            for a in range(nkt):
                nc.tensor.matmul(
                    sT[:, a * 128:(a + 1) * 128],
                    lhsT=khT[:, hl * T + (kt0 + a) * 128:hl * T + (kt0 + a + 1) * 128],
                    rhs=qhT[:, hl * T + qt * 128:hl * T + (qt + 1) * 128],
                    start=True,
                    stop=True,
                )
            pT = work.tile([128, 384], bf16, tag="pT")
            nc.scalar.activation(pT[:, :w], sT[:, :w], Exp)
            # band masks, post-exp on GpSimd: diagonal block keeps k<=q,
            # far block (qt-2) keeps k>q; middle block is fully in-window
            d0 = (nkt - 1) * 128
            nc.gpsimd.affine_select(
                pT[:, d0:d0 + 128], pT[:, d0:d0 + 128], pattern=[[1, 128]],
                compare_op=GE, fill=zero_reg, base=0, channel_multiplier=-1)
            if qt >= 2:
                nc.gpsimd.affine_select(
                    pT[:, 0:128], pT[:, 0:128], pattern=[[-1, 128]],
                    compare_op=GE, fill=zero_reg, base=-1, channel_multiplier=1)
            if pend is not None:
                pv(*pend)
            pend = (hl, pT)
        if qt >= 1:
            bcast(qt - 1)
        if qt >= 2:
            norm_muls(qt - 2)
            outproj(qt - 2)  # tensor work covering the last head's exp+mask
        pv(*pend)
        rs_sb = small.tile([1, 512], f32, tag="rs")
        nc.scalar.activation(rs_sb, o4[64:65, :], Copy)
        rc4s[qt] = small.tile([1, 512], f32, tag="rc4", name=f"rc4_{qt}")
        nc.vector.reciprocal_approx_fast(rc4s[qt], rs_sb)
    norm_muls(QT - 2)
    outproj(QT - 2)
    bcast(QT - 1)
    norm_muls(QT - 1)
    outproj(QT - 1)


def _build_program():
    import concourse.tile as tile
    from concourse import bacc, mybir

    bf16 = mybir.dt.bfloat16

    nc = bacc.Bacc("TRN2", target_bir_lowering=False, debug=False,
                   num_devices=N_CORES)
    aps = {
        "xT": nc.dram_tensor("xT", [128, 8 * T], bf16, kind="ExternalInput").ap(),
        "wT": nc.dram_tensor("wT", [128, 8 * 768], bf16, kind="ExternalInput").ap(),
        "woT": nc.dram_tensor("woT", [128, 2 * C], bf16, kind="ExternalInput").ap(),
        "cos4": nc.dram_tensor("cos4", [128, T], bf16, kind="ExternalInput").ap(),
        "sin4": nc.dram_tensor("sin4", [128, T], bf16, kind="ExternalInput").ap(),
        "y": nc.dram_tensor("y", [T, C], bf16, kind="ExternalOutput").ap(),
    }
    from contextlib import ExitStack

    with tile.TileContext(nc) as tc, ExitStack() as ctx:
        _emit(nc, tc, aps, ctx)
    nc.compile()
    return nc


def _get_program():
    global _PROGRAM
    if _PROGRAM is None:
        _PROGRAM = _build_program()
    return _PROGRAM


def _host_inputs(x, w_qkv, w_out):
    import ml_dtypes

    bf16 = ml_dtypes.bfloat16
    x = np.asarray(x, np.float32)
    w_qkv = np.asarray(w_qkv, np.float32)
    w_out = np.asarray(w_out, np.float32)

    wq, wk, wv = w_qkv[0:C], w_qkv[C:2 * C], w_qkv[2 * C:3 * C]
    scale = 1.0 / math.sqrt(DH)

    # RoPE tables (transposed, tiled over the 4 heads of a block)
    inv_freq = 1.0 / (10000.0 ** (np.arange(0, DH, 2, dtype=np.float32) / DH))
    freqs = np.outer(np.arange(T, dtype=np.float32), inv_freq)  # [T, 32]
    cos4 = np.ascontiguousarray(np.tile(np.cos(freqs).T, (4, 1))).astype(bf16)
    sin4 = np.ascontiguousarray(np.tile(np.sin(freqs).T, (4, 1))).astype(bf16)

    def ilv(m):  # [(kc*128), F] -> [128, kc*F] kc-major interleave
        kc = m.shape[0] // 128
        return np.ascontiguousarray(
            m.reshape(kc, 128, -1).transpose(1, 0, 2).reshape(128, -1))

    xT = [ilv(x[b].T).astype(bf16) for b in range(B)]

    in_maps = []
    for core in range(N_CORES):
        b, g = divmod(core, 4)
        hs = range(4 * g, 4 * g + 4)
        rows = []
        for half in range(2):  # q_x1, q_x2
            rows.append(np.concatenate(
                [wq[h * DH + 32 * half:h * DH + 32 * half + 32] for h in hs]) * scale)
        for half in range(2):  # k_x1, k_x2
            rows.append(np.concatenate(
                [wk[h * DH + 32 * half:h * DH + 32 * half + 32] for h in hs]))
        rows.append(wv[g * FQ:(g + 1) * FQ])
        wmat = np.concatenate(rows)  # [768, C]
        wT = ilv(wmat.T).astype(bf16)
        woT = ilv(w_out[:, g * FQ:(g + 1) * FQ].T).astype(bf16)
        in_maps.append({
            "xT": xT[b], "wT": wT, "woT": woT, "cos4": cos4, "sin4": sin4,
        })
    return in_maps


def kernel(x, w_qkv, w_out, _trace=False):
    from concourse import bass_utils

    nc = _get_program()
    in_maps = _host_inputs(x, w_qkv, w_out)
    res = bass_utils.run_bass_kernel_spmd(
        nc, in_maps, core_ids=list(range(N_CORES)), trace=_trace,
    )
    parts = [res.results[core]["y"].astype(np.float32) for core in range(N_CORES)]
    out = np.stack([
        parts[0] + parts[1] + parts[2] + parts[3],
        parts[4] + parts[5] + parts[6] + parts[7],
    ])
    if _trace:
        return out, res
    return out


# revision 24
# speedup vs baseline: 1.2491x; 1.0281x over previous
"""Sliding-window causal self-attention (B=2, T=2048, C=1024, H=16, Dh=64,
window=256) + QKV/out projections, sharded over 8 NeuronCores as
data-parallel over B (2) x tensor-parallel over head groups (4 heads/core).

Layout strategy: activations kept "token-transposed" (features on
partitions, tokens free) for the projections.  Attention scores are
computed TRANSPOSED (sT[k, q] = matmul(lhsT=k_tile, rhs=q_tile)) so that
exp(sT) is directly the P^T the PV matmul needs -- no PE transposes and no
diag-normalization matmuls.  Row sums come for free from a ones-column
appended to V inside the PV matmul; normalization is a reciprocal +
DMA-broadcast + one fused multiply into the attnT staging tile.
Band masking is applied post-exp with affine_select on the (idle) GpSimd
engine.  Input DMAs are chunked and ordered so the QKV matmuls start as
soon as the first weight/token chunks land, with PE-warmup matmuls
covering the HAM cold window during the DMA prologue.
"""

import math

import numpy as np

B = 2
T = 2048
C = 1024
H = 16
DH = 64
WINDOW = 256
HEADS_PER_CORE = 4
N_CORES = 8
QT = T // 128  # 16 query tiles of 128
FQ = HEADS_PER_CORE * DH  # 256 local features
VG = DH + 1  # v-block column group: 64 v columns + 1 ones column

_PROGRAM = None  # compile once per process


def _emit(nc, tc, aps, ctx):
    from concourse import mybir
    from concourse.bass import AP

    f32 = mybir.dt.float32
    bf16 = mybir.dt.bfloat16
    Exp = mybir.ActivationFunctionType.Exp
    Copy = mybir.ActivationFunctionType.Copy
    GE = mybir.AluOpType.is_ge

    xT, wT, woT, cos4, sin4, y = (
        aps["xT"], aps["wT"], aps["woT"], aps["cos4"], aps["sin4"], aps["y"],
    )

    consts = ctx.enter_context(tc.tile_pool(name="consts", bufs=1))
    stage = ctx.enter_context(tc.tile_pool(name="stage", bufs=1))
    work = ctx.enter_context(tc.tile_pool(name="work", bufs=3))
    tmp = ctx.enter_context(tc.tile_pool(name="tmp", bufs=4))
    small = ctx.enter_context(tc.tile_pool(name="small", bufs=4))
    ysbp = ctx.enter_context(tc.tile_pool(name="ysbp", bufs=2))
    pA = ctx.enter_context(tc.tile_pool(name="pA", bufs=2, space="PSUM"))
    pB = ctx.enter_context(tc.tile_pool(name="pB", bufs=2, space="PSUM"))
    pP = ctx.enter_context(tc.tile_pool(name="pP", bufs=1, space="PSUM"))
    pO = ctx.enter_context(tc.tile_pool(name="pO", bufs=3, space="PSUM"))
    p1 = [pA, pB, pP, pO]

    # ---- SBUF residents ----
    xT_sb = consts.tile([128, 8 * T], bf16, tag="xT")  # [C-part, (kc t)]
    wT_sb = consts.tile([128, 8 * 768], bf16, tag="wT")
    woT_sb = consts.tile([128, 2 * C], bf16, tag="woT")
    cos_sb = consts.tile([128, T], bf16, tag="cos")
    sin_sb = consts.tile([128, T], bf16, tag="sin")
    warm_sb = consts.tile([128, 640], bf16, tag="warm")

    rot = [stage.tile([128, T], bf16, tag=f"rot{i}", name=f"rot{i}")
           for i in range(4)]
    qhT = stage.tile([64, HEADS_PER_CORE * T], bf16, tag="qhT")
    khT = stage.tile([64, HEADS_PER_CORE * T], bf16, tag="khT")
    # v blocks: per key tile kt, per head: 64 v columns + a ones column
    v4 = stage.tile([128, QT * 4 * VG], bf16, tag="v4")
    attnT = stage.tile([128, 2 * T], bf16, tag="attnT")  # [(f%128), (kc t)]

    # ---- warmup / init (no DMA deps) ----
    nc.vector.memset(warm_sb, 0.0)
    v4v = v4.rearrange("p (kt h g) -> p kt h g", kt=QT, h=4)
    nc.gpsimd.memset(v4v[:, :, :, DH:DH + 1], 1.0)
    zero_reg = nc.gpsimd.to_reg(0.0)
    # touch every gpsimd ucode op once now: the library reload that pulls in
    # PartitionBroadcast/affine_select costs ~7us and must not hit mid-loop
    warm_g = stage.tile([64, 512], f32, tag="warmg")
    nc.gpsimd.memset(warm_g[0:1, :], 1.0)
    nc.gpsimd.partition_broadcast(warm_g, warm_g[0:1, :])
    warm_h = stage.tile([128, 128], bf16, tag="warmh")
    nc.gpsimd.memset(warm_h, 0.0)
    nc.gpsimd.affine_select(warm_h, warm_h, pattern=[[1, 128]],
                            compare_op=GE, fill=zero_reg, base=0,
                            channel_multiplier=-1)
    for i in range(36):  # keep the PE HAM-warm while input DMAs stream in
        acc = p1[i % 4].tile([128, 512], f32, tag="p")
        nc.tensor.matmul(acc, lhsT=warm_sb[:, :128], rhs=warm_sb[:, 128:640],
                         start=True, stop=True)

    # ---- input DMAs, in consumption order ----
    wT_v = wT_sb.rearrange("p (kc f) -> p kc f", kc=8)
    wT_in = wT.rearrange("p (kc f) -> p kc f", kc=8)
    nc.sync.dma_start(out=wT_v[:, :, 0:512], in_=wT_in[:, :, 0:512])
    xT_v = xT_sb.rearrange("p (kc t) -> p kc t", kc=8)
    xT_in = xT.rearrange("p (kc t) -> p kc t", kc=8)
    for s in range(4):
        tsl = slice(s * 512, (s + 1) * 512)
        nc.sync.dma_start(out=xT_v[:, :, tsl], in_=xT_in[:, :, tsl])
        nc.sync.dma_start(out=cos_sb[:, tsl], in_=cos4[:, tsl])
        nc.sync.dma_start(out=sin_sb[:, tsl], in_=sin4[:, tsl])
    nc.sync.dma_start(out=wT_v[:, :, 512:768], in_=wT_in[:, :, 512:768])
    nc.sync.dma_start(out=woT_sb, in_=woT)

    # ---- phase 1: q/k projection + RoPE (reads PSUM directly) ----
    for s in range(4):  # 512-token slices
        tsl = slice(s * 512, (s + 1) * 512)
        accs = []
        for blk in range(4):  # q_x1 q_x2 k_x1 k_x2
            acc = p1[blk].tile([128, 512], f32, tag="p")
            for kc in range(8):
                nc.tensor.matmul(
                    acc,
                    lhsT=wT_sb[:, kc * 768 + blk * 128:kc * 768 + (blk + 1) * 128],
                    rhs=xT_sb[:, kc * T + s * 512:kc * T + (s + 1) * 512],
                    start=(kc == 0),
                    stop=(kc == 7),
                )
            accs.append(acc)
        # RoPE: rot1 = x1*cos - x2*sin ; rot2 = x2*cos + x1*sin
        for pair in range(2):  # 0 -> q, 1 -> k
            x1, x2 = accs[2 * pair], accs[2 * pair + 1]
            r1, r2 = rot[2 * pair][:, tsl], rot[2 * pair + 1][:, tsl]
            t1 = tmp.tile([128, 512], bf16, tag="t1")
            t2 = tmp.tile([128, 512], bf16, tag="t2")
            t3 = tmp.tile([128, 512], bf16, tag="t3")
            t4 = tmp.tile([128, 512], bf16, tag="t4")
            nc.vector.tensor_mul(t1, x1, cos_sb[:, tsl])
            nc.vector.tensor_mul(t2, x2, sin_sb[:, tsl])
            nc.vector.tensor_sub(r1, t1, t2)
            nc.vector.tensor_mul(t3, x2, cos_sb[:, tsl])
            nc.vector.tensor_mul(t4, x1, sin_sb[:, tsl])
            nc.vector.tensor_add(r2, t3, t4)
        # repack this token slice into head-contiguous [64, (h t)] layout
        for hl in range(HEADS_PER_CORE):
            hsl = slice(hl * T + s * 512, hl * T + (s + 1) * 512)
            for half in range(2):
                psl = slice(half * 32, (half + 1) * 32)
                rsl = slice(hl * 32, (hl + 1) * 32)
                nc.sync.dma_start(out=qhT[psl, hsl], in_=rot[half][rsl, tsl])
                nc.sync.dma_start(out=khT[psl, hsl], in_=rot[2 + half][rsl, tsl])

    # ---- phase 2: software-pipelined attention ----
    # per iteration i: v(i), bcast(i-1), attention(i)+recip(i),
    # normalize-muls(i-1), out-proj(i-1).  The tensor queue never waits on
    # the reciprocal: by the time it reaches bcast(i-1), rc4(i-1) is old.
    o4s = [None] * QT
    rc4s = [None] * QT
    rcbs = [None] * QT

    def bcast(qt):  # rcb[d, (h q)] = 1/rowsum broadcast over partitions
        # gpsimd, emitted after a head-loop's masks: it executes in the
        # mask-free window while PV3/outproj run, never blocking a mask
        rcbs[qt] = work.tile([64, 512], f32, tag="rcb", name=f"rcb{qt}")
        nc.gpsimd.partition_broadcast(rcbs[qt], rc4s[qt])

    def norm_muls(qt):  # attnT <- o4 * rcb  (both PSUM)
        ov = o4s[qt][0:64, :].rearrange("p (h q) -> p h q", h=4)
        rv = rcbs[qt].rearrange("p (h q) -> p h q", h=4)
        av = attnT.rearrange("p (b t) -> p b t", b=2)
        qsl = slice(qt * 128, (qt + 1) * 128)
        nc.vector.tensor_mul(av[0:64, :, qsl], ov[:, 0::2, :], rv[:, 0::2, :])
        nc.vector.tensor_mul(av[64:128, :, qsl], ov[:, 1::2, :], rv[:, 1::2, :])

    def outproj(qt):
        ysb = ysbp.tile([128, C], bf16, tag="ysb")
        for nh in range(2):
            acc = (pA if nh == 0 else pB).tile([128, 512], f32, tag="p")
            for kc in range(2):
                nc.tensor.matmul(
                    acc,
                    lhsT=attnT[:, kc * T + qt * 128:kc * T + (qt + 1) * 128],
                    rhs=woT_sb[:, kc * C + nh * 512:kc * C + (nh + 1) * 512],
                    start=(kc == 0),
                    stop=(kc == 1),
                )
            if nh == 0:  # vector is idle mid-iteration; scalar still on exps
                nc.vector.tensor_copy(ysb[:, nh * 512:(nh + 1) * 512], acc)
            else:
                nc.scalar.copy(ysb[:, nh * 512:(nh + 1) * 512], acc)
        nc.sync.dma_start(out=y[qt * 128:(qt + 1) * 128, :], in_=ysb)

    for qt in range(QT):
        # v tile qt in natural (token-partition) layout, strided into v4
        acc = pA.tile([128, FQ], f32, tag="p")
        for kc in range(8):
            nc.tensor.matmul(
                acc,
                lhsT=xT_sb[:, kc * T + qt * 128:kc * T + (qt + 1) * 128],
                rhs=wT_sb[:, kc * 768 + 512:kc * 768 + 768],
                start=(kc == 0),
                stop=(kc == 7),
            )
        nc.any.tensor_copy(
            v4v[:, qt, :, 0:DH],
            acc.rearrange("p (h f) -> p h f", h=4),
        )
        if qt >= 2:
            bcast(qt - 2)

        nkt = min(qt + 1, 3)  # key tiles in window
        w = 128 * nkt
        kt0 = max(qt - 2, 0)
        o4 = o4s[qt] = pO.tile([65, 512], f32, tag="p", name=f"o4_{qt}")

        def pv(hl, pT):
            for a in range(nkt):
                kt = kt0 + a
                nc.tensor.matmul(
                    o4[:, hl * 128:(hl + 1) * 128],
                    lhsT=v4[:, (kt * 4 + hl) * VG:(kt * 4 + hl + 1) * VG],
                    rhs=pT[:, a * 128:(a + 1) * 128],
                    start=(a == 0),
                    stop=(a == nkt - 1),
                )

        pend = None  # (hl, pT) whose PV is issued under the NEXT head
        for hl in range(HEADS_PER_CORE):
            # transposed scores sT[k, q] accumulate straight into one bank
            sT = pB.tile([128, 384], f32, tag="p")
            for a in range(nkt):
                nc.tensor.matmul(
                    sT[:, a * 128:(a + 1) * 128],
                    lhsT=khT[:, hl * T + (kt0 + a) * 128:hl * T + (kt0 + a + 1) * 128],
                    rhs=qhT[:, hl * T + qt * 128:hl * T + (qt + 1) * 128],
                    start=True,
                    stop=True,
                )
            pT = work.tile([128, 384], bf16, tag="pT")
            nc.scalar.activation(pT[:, :w], sT[:, :w], Exp)
            # band masks, post-exp on GpSimd: diagonal block keeps k<=q,
            # far block (qt-2) keeps k>q; middle block is fully in-window
            d0 = (nkt - 1) * 128
            nc.gpsimd.affine_select(
                pT[:, d0:d0 + 128], pT[:, d0:d0 + 128], pattern=[[1, 128]],
                compare_op=GE, fill=zero_reg, base=0, channel_multiplier=-1)
            if qt >= 2:
                nc.gpsimd.affine_select(
                    pT[:, 0:128], pT[:, 0:128], pattern=[[-1, 128]],
                    compare_op=GE, fill=zero_reg, base=-1, channel_multiplier=1)
            if pend is not None:
                pv(*pend)
            pend = (hl, pT)
        if qt >= 2:
            norm_muls(qt - 2)
            outproj(qt - 2)  # tensor work covering the last head's exp+mask
        pv(*pend)
        rs_sb = small.tile([1, 512], f32, tag="rs")
        nc.scalar.activation(rs_sb, o4[64:65, :], Copy)
        rc4s[qt] = small.tile([1, 512], f32, tag="rc4", name=f"rc4_{qt}")
        nc.vector.reciprocal_approx_fast(rc4s[qt], rs_sb)
    for qt in (QT - 2, QT - 1):
        bcast(qt)
        norm_muls(qt)
        outproj(qt)


def _build_program():
    import concourse.tile as tile
    from concourse import bacc, mybir

    bf16 = mybir.dt.bfloat16

    nc = bacc.Bacc("TRN2", target_bir_lowering=False, debug=False,
                   num_devices=N_CORES)
    aps = {
        "xT": nc.dram_tensor("xT", [128, 8 * T], bf16, kind="ExternalInput").ap(),
        "wT": nc.dram_tensor("wT", [128, 8 * 768], bf16, kind="ExternalInput").ap(),
        "woT": nc.dram_tensor("woT", [128, 2 * C], bf16, kind="ExternalInput").ap(),
        "cos4": nc.dram_tensor("cos4", [128, T], bf16, kind="ExternalInput").ap(),
        "sin4": nc.dram_tensor("sin4", [128, T], bf16, kind="ExternalInput").ap(),
        "y": nc.dram_tensor("y", [T, C], bf16, kind="ExternalOutput").ap(),
    }
    from contextlib import ExitStack

    with tile.TileContext(nc) as tc, ExitStack() as ctx:
        _emit(nc, tc, aps, ctx)
    nc.compile()
    return nc


def _get_program():
    global _PROGRAM
    if _PROGRAM is None:
        _PROGRAM = _build_program()
    return _PROGRAM


def _host_inputs(x, w_qkv, w_out):
    import ml_dtypes

    bf16 = ml_dtypes.bfloat16
    x = np.asarray(x, np.float32)
    w_qkv = np.asarray(w_qkv, np.float32)
    w_out = np.asarray(w_out, np.float32)

    wq, wk, wv = w_qkv[0:C], w_qkv[C:2 * C], w_qkv[2 * C:3 * C]
    scale = 1.0 / math.sqrt(DH)

    # RoPE tables (transposed, tiled over the 4 heads of a block)
    inv_freq = 1.0 / (10000.0 ** (np.arange(0, DH, 2, dtype=np.float32) / DH))
    freqs = np.outer(np.arange(T, dtype=np.float32), inv_freq)  # [T, 32]
    cos4 = np.ascontiguousarray(np.tile(np.cos(freqs).T, (4, 1))).astype(bf16)
    sin4 = np.ascontiguousarray(np.tile(np.sin(freqs).T, (4, 1))).astype(bf16)

    def ilv(m):  # [(kc*128), F] -> [128, kc*F] kc-major interleave
        kc = m.shape[0] // 128
        return np.ascontiguousarray(
            m.reshape(kc, 128, -1).transpose(1, 0, 2).reshape(128, -1))

    xT = [ilv(x[b].T).astype(bf16) for b in range(B)]

    in_maps = []
    for core in range(N_CORES):
        b, g = divmod(core, 4)
        hs = range(4 * g, 4 * g + 4)
        rows = []
        for half in range(2):  # q_x1, q_x2
            rows.append(np.concatenate(
                [wq[h * DH + 32 * half:h * DH + 32 * half + 32] for h in hs]) * scale)
        for half in range(2):  # k_x1, k_x2
            rows.append(np.concatenate(
                [wk[h * DH + 32 * half:h * DH + 32 * half + 32] for h in hs]))
        rows.append(wv[g * FQ:(g + 1) * FQ])
        wmat = np.concatenate(rows)  # [768, C]
        wT = ilv(wmat.T).astype(bf16)
        woT = ilv(w_out[:, g * FQ:(g + 1) * FQ].T).astype(bf16)
        in_maps.append({
            "xT": xT[b], "wT": wT, "woT": woT, "cos4": cos4, "sin4": sin4,
        })
    return in_maps


def kernel(x, w_qkv, w_out, _trace=False):
    from concourse import bass_utils

    nc = _get_program()
    in_maps = _host_inputs(x, w_qkv, w_out)
    res = bass_utils.run_bass_kernel_spmd(
        nc, in_maps, core_ids=list(range(N_CORES)), trace=_trace,
    )
    parts = [res.results[core]["y"].astype(np.float32) for core in range(N_CORES)]
    out = np.stack([
        parts[0] + parts[1] + parts[2] + parts[3],
        parts[4] + parts[5] + parts[6] + parts[7],
    ])
    if _trace:
        return out, res
    return out


# revision 25
# speedup vs baseline: 1.2507x; 1.0013x over previous
"""Sliding-window causal self-attention (B=2, T=2048, C=1024, H=16, Dh=64,
window=256) + QKV/out projections, sharded over 8 NeuronCores as
data-parallel over B (2) x tensor-parallel over head groups (4 heads/core).

Layout strategy: activations kept "token-transposed" (features on
partitions, tokens free) for the projections.  Attention scores are
computed TRANSPOSED (sT[k, q] = matmul(lhsT=k_tile, rhs=q_tile)) so that
exp(sT) is directly the P^T the PV matmul needs -- no PE transposes and no
diag-normalization matmuls.  Row sums come for free from a ones-column
appended to V inside the PV matmul; normalization is a reciprocal +
DMA-broadcast + one fused multiply into the attnT staging tile.
Band masking is applied post-exp with affine_select on the (idle) GpSimd
engine.  Input DMAs are chunked and ordered so the QKV matmuls start as
soon as the first weight/token chunks land, with PE-warmup matmuls
covering the HAM cold window during the DMA prologue.
"""

import math

import numpy as np

B = 2
T = 2048
C = 1024
H = 16
DH = 64
WINDOW = 256
HEADS_PER_CORE = 4
N_CORES = 8
QT = T // 128  # 16 query tiles of 128
FQ = HEADS_PER_CORE * DH  # 256 local features
VG = DH + 1  # v-block column group: 64 v columns + 1 ones column

_PROGRAM = None  # compile once per process


def _emit(nc, tc, aps, ctx):
    from concourse import mybir
    from concourse.bass import AP

    f32 = mybir.dt.float32
    bf16 = mybir.dt.bfloat16
    Exp = mybir.ActivationFunctionType.Exp
    Copy = mybir.ActivationFunctionType.Copy
    GE = mybir.AluOpType.is_ge

    xT, wT, woT, cos4, sin4, y = (
        aps["xT"], aps["wT"], aps["woT"], aps["cos4"], aps["sin4"], aps["y"],
    )

    consts = ctx.enter_context(tc.tile_pool(name="consts", bufs=1))
    stage = ctx.enter_context(tc.tile_pool(name="stage", bufs=1))
    work = ctx.enter_context(tc.tile_pool(name="work", bufs=3))
    tmp = ctx.enter_context(tc.tile_pool(name="tmp", bufs=4))
    small = ctx.enter_context(tc.tile_pool(name="small", bufs=4))
    ysbp = ctx.enter_context(tc.tile_pool(name="ysbp", bufs=2))
    pA = ctx.enter_context(tc.tile_pool(name="pA", bufs=2, space="PSUM"))
    pB = ctx.enter_context(tc.tile_pool(name="pB", bufs=2, space="PSUM"))
    pP = ctx.enter_context(tc.tile_pool(name="pP", bufs=1, space="PSUM"))
    pO = ctx.enter_context(tc.tile_pool(name="pO", bufs=3, space="PSUM"))
    p1 = [pA, pB, pP, pO]

    # ---- SBUF residents ----
    xT_sb = consts.tile([128, 8 * T], bf16, tag="xT")  # [C-part, (kc t)]
    wT_sb = consts.tile([128, 8 * 768], bf16, tag="wT")
    woT_sb = consts.tile([128, 2 * C], bf16, tag="woT")
    cos_sb = consts.tile([128, T], bf16, tag="cos")
    sin_sb = consts.tile([128, T], bf16, tag="sin")
    warm_sb = consts.tile([128, 640], bf16, tag="warm")

    rot = [stage.tile([128, T], bf16, tag=f"rot{i}", name=f"rot{i}")
           for i in range(4)]
    qhT = stage.tile([64, HEADS_PER_CORE * T], bf16, tag="qhT")
    khT = stage.tile([64, HEADS_PER_CORE * T], bf16, tag="khT")
    # v blocks: per key tile kt, per head: 64 v columns + a ones column
    v4 = stage.tile([128, QT * 4 * VG], bf16, tag="v4")
    attnT = stage.tile([128, 2 * T], bf16, tag="attnT")  # [(f%128), (kc t)]

    # ---- warmup / init (no DMA deps) ----
    nc.vector.memset(warm_sb, 0.0)
    v4v = v4.rearrange("p (kt h g) -> p kt h g", kt=QT, h=4)
    nc.gpsimd.memset(v4v[:, :, :, DH:DH + 1], 1.0)
    zero_reg = nc.gpsimd.to_reg(0.0)
    # touch every gpsimd ucode op once now: the library reload that pulls in
    # PartitionBroadcast/affine_select costs ~7us and must not hit mid-loop
    warm_g = stage.tile([64, 512], f32, tag="warmg")
    nc.gpsimd.memset(warm_g[0:1, :], 1.0)
    nc.gpsimd.partition_broadcast(warm_g, warm_g[0:1, :])
    warm_h = stage.tile([128, 128], bf16, tag="warmh")
    nc.gpsimd.memset(warm_h, 0.0)
    nc.gpsimd.affine_select(warm_h, warm_h, pattern=[[1, 128]],
                            compare_op=GE, fill=zero_reg, base=0,
                            channel_multiplier=-1)
    for i in range(36):  # keep the PE HAM-warm while input DMAs stream in
        acc = p1[i % 4].tile([128, 512], f32, tag="p")
        nc.tensor.matmul(acc, lhsT=warm_sb[:, :128], rhs=warm_sb[:, 128:640],
                         start=True, stop=True)

    # ---- input DMAs, in consumption order ----
    wT_v = wT_sb.rearrange("p (kc f) -> p kc f", kc=8)
    wT_in = wT.rearrange("p (kc f) -> p kc f", kc=8)
    nc.sync.dma_start(out=wT_v[:, :, 0:512], in_=wT_in[:, :, 0:512])
    xT_v = xT_sb.rearrange("p (kc t) -> p kc t", kc=8)
    xT_in = xT.rearrange("p (kc t) -> p kc t", kc=8)
    for s in range(4):
        tsl = slice(s * 512, (s + 1) * 512)
        nc.sync.dma_start(out=xT_v[:, :, tsl], in_=xT_in[:, :, tsl])
        nc.sync.dma_start(out=cos_sb[:, tsl], in_=cos4[:, tsl])
        nc.sync.dma_start(out=sin_sb[:, tsl], in_=sin4[:, tsl])
    nc.sync.dma_start(out=wT_v[:, :, 512:768], in_=wT_in[:, :, 512:768])
    nc.sync.dma_start(out=woT_sb, in_=woT)

    # ---- phase 1: q/k projection + RoPE (reads PSUM directly) ----
    for s in range(4):  # 512-token slices
        tsl = slice(s * 512, (s + 1) * 512)
        accs = []
        for blk in range(4):  # q_x1 q_x2 k_x1 k_x2
            acc = p1[blk].tile([128, 512], f32, tag="p")
            for kc in range(8):
                nc.tensor.matmul(
                    acc,
                    lhsT=wT_sb[:, kc * 768 + blk * 128:kc * 768 + (blk + 1) * 128],
                    rhs=xT_sb[:, kc * T + s * 512:kc * T + (s + 1) * 512],
                    start=(kc == 0),
                    stop=(kc == 7),
                )
            accs.append(acc)
        # RoPE: rot1 = x1*cos - x2*sin ; rot2 = x2*cos + x1*sin
        for pair in range(2):  # 0 -> q, 1 -> k
            x1, x2 = accs[2 * pair], accs[2 * pair + 1]
            r1, r2 = rot[2 * pair][:, tsl], rot[2 * pair + 1][:, tsl]
            t1 = tmp.tile([128, 512], bf16, tag="t1")
            t2 = tmp.tile([128, 512], bf16, tag="t2")
            t3 = tmp.tile([128, 512], bf16, tag="t3")
            t4 = tmp.tile([128, 512], bf16, tag="t4")
            nc.vector.tensor_mul(t1, x1, cos_sb[:, tsl])
            nc.vector.tensor_mul(t2, x2, sin_sb[:, tsl])
            nc.vector.tensor_sub(r1, t1, t2)
            nc.vector.tensor_mul(t3, x2, cos_sb[:, tsl])
            nc.vector.tensor_mul(t4, x1, sin_sb[:, tsl])
            nc.vector.tensor_add(r2, t3, t4)
        # repack this token slice into head-contiguous [64, (h t)] layout
        for hl in range(HEADS_PER_CORE):
            hsl = slice(hl * T + s * 512, hl * T + (s + 1) * 512)
            for half in range(2):
                psl = slice(half * 32, (half + 1) * 32)
                rsl = slice(hl * 32, (hl + 1) * 32)
                nc.sync.dma_start(out=qhT[psl, hsl], in_=rot[half][rsl, tsl])
                nc.sync.dma_start(out=khT[psl, hsl], in_=rot[2 + half][rsl, tsl])

    # ---- phase 2: software-pipelined attention ----
    # per iteration i: v(i), bcast(i-1), attention(i)+recip(i),
    # normalize-muls(i-1), out-proj(i-1).  The tensor queue never waits on
    # the reciprocal: by the time it reaches bcast(i-1), rc4(i-1) is old.
    o4s = [None] * QT
    rc4s = [None] * QT
    rcbs = [None] * QT

    def bcast(qt):  # rcb[d, (h q)] = 1/rowsum broadcast over partitions
        # gpsimd, emitted after a head-loop's masks: it executes in the
        # mask-free window while PV3/outproj run, never blocking a mask
        rcbs[qt] = work.tile([64, 512], f32, tag="rcb", name=f"rcb{qt}")
        nc.gpsimd.partition_broadcast(rcbs[qt], rc4s[qt])

    def norm_muls(qt):  # attnT <- o4 * rcb  (both PSUM)
        ov = o4s[qt][0:64, :].rearrange("p (h q) -> p h q", h=4)
        rv = rcbs[qt].rearrange("p (h q) -> p h q", h=4)
        av = attnT.rearrange("p (b t) -> p b t", b=2)
        qsl = slice(qt * 128, (qt + 1) * 128)
        nc.vector.tensor_mul(av[0:64, :, qsl], ov[:, 0::2, :], rv[:, 0::2, :])
        nc.vector.tensor_mul(av[64:128, :, qsl], ov[:, 1::2, :], rv[:, 1::2, :])

    def outproj(qt):
        ysb = ysbp.tile([128, C], bf16, tag="ysb")
        for nh in range(2):
            acc = (pA if nh == 0 else pB).tile([128, 512], f32, tag="p")
            for kc in range(2):
                nc.tensor.matmul(
                    acc,
                    lhsT=attnT[:, kc * T + qt * 128:kc * T + (qt + 1) * 128],
                    rhs=woT_sb[:, kc * C + nh * 512:kc * C + (nh + 1) * 512],
                    start=(kc == 0),
                    stop=(kc == 1),
                )
            if nh == 0:  # vector is idle mid-iteration; scalar still on exps
                nc.vector.tensor_copy(ysb[:, nh * 512:(nh + 1) * 512], acc)
            else:
                nc.scalar.copy(ysb[:, nh * 512:(nh + 1) * 512], acc)
        nc.sync.dma_start(out=y[qt * 128:(qt + 1) * 128, :], in_=ysb)

    for qt in range(QT):
        # v tile qt in natural (token-partition) layout, strided into v4
        acc = pA.tile([128, FQ], f32, tag="p")
        for kc in range(8):
            nc.tensor.matmul(
                acc,
                lhsT=xT_sb[:, kc * T + qt * 128:kc * T + (qt + 1) * 128],
                rhs=wT_sb[:, kc * 768 + 512:kc * 768 + 768],
                start=(kc == 0),
                stop=(kc == 7),
            )
        nc.any.tensor_copy(
            v4v[:, qt, :, 0:DH],
            acc.rearrange("p (h f) -> p h f", h=4),
        )
        if qt >= 2:
            bcast(qt - 2)

        nkt = min(qt + 1, 3)  # key tiles in window
        w = 128 * nkt
        kt0 = max(qt - 2, 0)
        o4 = o4s[qt] = pO.tile([65, 512], f32, tag="p", name=f"o4_{qt}")

        def pv(hl, pT):
            for a in range(nkt):
                kt = kt0 + a
                nc.tensor.matmul(
                    o4[:, hl * 128:(hl + 1) * 128],
                    lhsT=v4[:, (kt * 4 + hl) * VG:(kt * 4 + hl + 1) * VG],
                    rhs=pT[:, a * 128:(a + 1) * 128],
                    start=(a == 0),
                    stop=(a == nkt - 1),
                )

        pend = []  # (hl, pT) whose PV issues two heads later
        for hl in range(HEADS_PER_CORE):
            # transposed scores sT[k, q] accumulate straight into one bank
            sT = pB.tile([128, 384], f32, tag="p")
            for a in range(nkt):
                nc.tensor.matmul(
                    sT[:, a * 128:(a + 1) * 128],
                    lhsT=khT[:, hl * T + (kt0 + a) * 128:hl * T + (kt0 + a + 1) * 128],
                    rhs=qhT[:, hl * T + qt * 128:hl * T + (qt + 1) * 128],
                    start=True,
                    stop=True,
                )
            pT = work.tile([128, 384], bf16, tag="pT")
            nc.scalar.activation(pT[:, :w], sT[:, :w], Exp)
            # band masks, post-exp on GpSimd: diagonal block keeps k<=q,
            # far block (qt-2) keeps k>q; middle block is fully in-window
            d0 = (nkt - 1) * 128
            nc.gpsimd.affine_select(
                pT[:, d0:d0 + 128], pT[:, d0:d0 + 128], pattern=[[1, 128]],
                compare_op=GE, fill=zero_reg, base=0, channel_multiplier=-1)
            if qt >= 2:
                nc.gpsimd.affine_select(
                    pT[:, 0:128], pT[:, 0:128], pattern=[[-1, 128]],
                    compare_op=GE, fill=zero_reg, base=-1, channel_multiplier=1)
            if len(pend) >= 2:
                pv(*pend.pop(0))
            pend.append((hl, pT))
        pv(*pend.pop(0))
        if qt >= 2:
            norm_muls(qt - 2)
            outproj(qt - 2)  # tensor work covering the last head's exp+mask
        pv(*pend.pop(0))
        rs_sb = small.tile([1, 512], f32, tag="rs")
        nc.scalar.activation(rs_sb, o4[64:65, :], Copy)
        rc4s[qt] = small.tile([1, 512], f32, tag="rc4", name=f"rc4_{qt}")
        nc.vector.reciprocal_approx_fast(rc4s[qt], rs_sb)
    for qt in (QT - 2, QT - 1):
        bcast(qt)
        norm_muls(qt)
        outproj(qt)


def _build_program():
    import concourse.tile as tile
    from concourse import bacc, mybir

    bf16 = mybir.dt.bfloat16

    nc = bacc.Bacc("TRN2", target_bir_lowering=False, debug=False,
                   num_devices=N_CORES)
    aps = {
        "xT": nc.dram_tensor("xT", [128, 8 * T], bf16, kind="ExternalInput").ap(),
        "wT": nc.dram_tensor("wT", [128, 8 * 768], bf16, kind="ExternalInput").ap(),
        "woT": nc.dram_tensor("woT", [128, 2 * C], bf16, kind="ExternalInput").ap(),
        "cos4": nc.dram_tensor("cos4", [128, T], bf16, kind="ExternalInput").ap(),
        "sin4": nc.dram_tensor("sin4", [128, T], bf16, kind="ExternalInput").ap(),
        "y": nc.dram_tensor("y", [T, C], bf16, kind="ExternalOutput").ap(),
    }
    from contextlib import ExitStack

    with tile.TileContext(nc) as tc, ExitStack() as ctx:
        _emit(nc, tc, aps, ctx)
    nc.compile()
    return nc


def _get_program():
    global _PROGRAM
    if _PROGRAM is None:
        _PROGRAM = _build_program()
    return _PROGRAM


def _host_inputs(x, w_qkv, w_out):
    import ml_dtypes

    bf16 = ml_dtypes.bfloat16
    x = np.asarray(x, np.float32)
    w_qkv = np.asarray(w_qkv, np.float32)
    w_out = np.asarray(w_out, np.float32)

    wq, wk, wv = w_qkv[0:C], w_qkv[C:2 * C], w_qkv[2 * C:3 * C]
    scale = 1.0 / math.sqrt(DH)

    # RoPE tables (transposed, tiled over the 4 heads of a block)
    inv_freq = 1.0 / (10000.0 ** (np.arange(0, DH, 2, dtype=np.float32) / DH))
    freqs = np.outer(np.arange(T, dtype=np.float32), inv_freq)  # [T, 32]
    cos4 = np.ascontiguousarray(np.tile(np.cos(freqs).T, (4, 1))).astype(bf16)
    sin4 = np.ascontiguousarray(np.tile(np.sin(freqs).T, (4, 1))).astype(bf16)

    def ilv(m):  # [(kc*128), F] -> [128, kc*F] kc-major interleave
        kc = m.shape[0] // 128
        return np.ascontiguousarray(
            m.reshape(kc, 128, -1).transpose(1, 0, 2).reshape(128, -1))

    xT = [ilv(x[b].T).astype(bf16) for b in range(B)]

    in_maps = []
    for core in range(N_CORES):
        b, g = divmod(core, 4)
        hs = range(4 * g, 4 * g + 4)
        rows = []
        for half in range(2):  # q_x1, q_x2
            rows.append(np.concatenate(
                [wq[h * DH + 32 * half:h * DH + 32 * half + 32] for h in hs]) * scale)
        for half in range(2):  # k_x1, k_x2
            rows.append(np.concatenate(
                [wk[h * DH + 32 * half:h * DH + 32 * half + 32] for h in hs]))
        rows.append(wv[g * FQ:(g + 1) * FQ])
        wmat = np.concatenate(rows)  # [768, C]
        wT = ilv(wmat.T).astype(bf16)
        woT = ilv(w_out[:, g * FQ:(g + 1) * FQ].T).astype(bf16)
        in_maps.append({
            "xT": xT[b], "wT": wT, "woT": woT, "cos4": cos4, "sin4": sin4,
        })
    return in_maps


def kernel(x, w_qkv, w_out, _trace=False):
    from concourse import bass_utils

    nc = _get_program()
    in_maps = _host_inputs(x, w_qkv, w_out)
    res = bass_utils.run_bass_kernel_spmd(
        nc, in_maps, core_ids=list(range(N_CORES)), trace=_trace,
    )
    parts = [res.results[core]["y"].astype(np.float32) for core in range(N_CORES)]
    out = np.stack([
        parts[0] + parts[1] + parts[2] + parts[3],
        parts[4] + parts[5] + parts[6] + parts[7],
    ])
    if _trace:
        return out, res
    return out


# revision 26
# speedup vs baseline: 1.2589x; 1.0065x over previous
"""Sliding-window causal self-attention (B=2, T=2048, C=1024, H=16, Dh=64,
window=256) + QKV/out projections, sharded over 8 NeuronCores as
data-parallel over B (2) x tensor-parallel over head groups (4 heads/core).

Layout strategy: activations kept "token-transposed" (features on
partitions, tokens free) for the projections.  Attention scores are
computed TRANSPOSED (sT[k, q] = matmul(lhsT=k_tile, rhs=q_tile)) so that
exp(sT) is directly the P^T the PV matmul needs -- no PE transposes and no
diag-normalization matmuls.  Row sums come for free from a ones-column
appended to V inside the PV matmul; normalization is a reciprocal +
DMA-broadcast + one fused multiply into the attnT staging tile.
Band masking is applied post-exp with affine_select on the (idle) GpSimd
engine.  Input DMAs are chunked and ordered so the QKV matmuls start as
soon as the first weight/token chunks land, with PE-warmup matmuls
covering the HAM cold window during the DMA prologue.
"""

import math

import numpy as np

B = 2
T = 2048
C = 1024
H = 16
DH = 64
WINDOW = 256
HEADS_PER_CORE = 4
N_CORES = 8
QT = T // 128  # 16 query tiles of 128
FQ = HEADS_PER_CORE * DH  # 256 local features
VG = DH + 1  # v-block column group: 64 v columns + 1 ones column

_PROGRAM = None  # compile once per process


def _emit(nc, tc, aps, ctx):
    from concourse import mybir
    from concourse.bass import AP

    f32 = mybir.dt.float32
    bf16 = mybir.dt.bfloat16
    Exp = mybir.ActivationFunctionType.Exp
    Copy = mybir.ActivationFunctionType.Copy
    GE = mybir.AluOpType.is_ge

    xT, wT, woT, cos4, sin4, y = (
        aps["xT"], aps["wT"], aps["woT"], aps["cos4"], aps["sin4"], aps["y"],
    )

    consts = ctx.enter_context(tc.tile_pool(name="consts", bufs=1))
    stage = ctx.enter_context(tc.tile_pool(name="stage", bufs=1))
    work = ctx.enter_context(tc.tile_pool(name="work", bufs=4))
    tmp = ctx.enter_context(tc.tile_pool(name="tmp", bufs=4))
    small = ctx.enter_context(tc.tile_pool(name="small", bufs=4))
    ysbp = ctx.enter_context(tc.tile_pool(name="ysbp", bufs=2))
    pA = ctx.enter_context(tc.tile_pool(name="pA", bufs=2, space="PSUM"))
    pB = ctx.enter_context(tc.tile_pool(name="pB", bufs=2, space="PSUM"))
    pP = ctx.enter_context(tc.tile_pool(name="pP", bufs=1, space="PSUM"))
    pO = ctx.enter_context(tc.tile_pool(name="pO", bufs=3, space="PSUM"))
    p1 = [pA, pB, pP, pO]

    # ---- SBUF residents ----
    xT_sb = consts.tile([128, 8 * T], bf16, tag="xT")  # [C-part, (kc t)]
    wT_sb = consts.tile([128, 8 * 768], bf16, tag="wT")
    woT_sb = consts.tile([128, 2 * C], bf16, tag="woT")
    cos_sb = consts.tile([128, T], bf16, tag="cos")
    sin_sb = consts.tile([128, T], bf16, tag="sin")
    warm_sb = consts.tile([128, 640], bf16, tag="warm")

    rot = [stage.tile([128, T], bf16, tag=f"rot{i}", name=f"rot{i}")
           for i in range(4)]
    qhT = stage.tile([64, HEADS_PER_CORE * T], bf16, tag="qhT")
    khT = stage.tile([64, HEADS_PER_CORE * T], bf16, tag="khT")
    # v blocks: per key tile kt, per head: 64 v columns + a ones column
    v4 = stage.tile([128, QT * 4 * VG], bf16, tag="v4")
    attnT = stage.tile([128, 2 * T], bf16, tag="attnT")  # [(f%128), (kc t)]

    # ---- warmup / init (no DMA deps) ----
    nc.vector.memset(warm_sb, 0.0)
    v4v = v4.rearrange("p (kt h g) -> p kt h g", kt=QT, h=4)
    nc.gpsimd.memset(v4v[:, :, :, DH:DH + 1], 1.0)
    zero_reg = nc.gpsimd.to_reg(0.0)
    # touch every gpsimd ucode op once now: the library reload that pulls in
    # PartitionBroadcast/affine_select costs ~7us and must not hit mid-loop
    warm_g = stage.tile([64, 512], f32, tag="warmg")
    nc.gpsimd.memset(warm_g[0:1, :], 1.0)
    nc.gpsimd.partition_broadcast(warm_g, warm_g[0:1, :])
    warm_h = stage.tile([128, 128], bf16, tag="warmh")
    nc.gpsimd.memset(warm_h, 0.0)
    nc.gpsimd.affine_select(warm_h, warm_h, pattern=[[1, 128]],
                            compare_op=GE, fill=zero_reg, base=0,
                            channel_multiplier=-1)
    for i in range(36):  # keep the PE HAM-warm while input DMAs stream in
        acc = p1[i % 4].tile([128, 512], f32, tag="p")
        nc.tensor.matmul(acc, lhsT=warm_sb[:, :128], rhs=warm_sb[:, 128:640],
                         start=True, stop=True)

    # ---- input DMAs, in consumption order ----
    wT_v = wT_sb.rearrange("p (kc f) -> p kc f", kc=8)
    wT_in = wT.rearrange("p (kc f) -> p kc f", kc=8)
    nc.sync.dma_start(out=wT_v[:, :, 0:512], in_=wT_in[:, :, 0:512])
    xT_v = xT_sb.rearrange("p (kc t) -> p kc t", kc=8)
    xT_in = xT.rearrange("p (kc t) -> p kc t", kc=8)
    for s in range(4):
        tsl = slice(s * 512, (s + 1) * 512)
        nc.sync.dma_start(out=xT_v[:, :, tsl], in_=xT_in[:, :, tsl])
        nc.sync.dma_start(out=cos_sb[:, tsl], in_=cos4[:, tsl])
        nc.sync.dma_start(out=sin_sb[:, tsl], in_=sin4[:, tsl])
    nc.sync.dma_start(out=wT_v[:, :, 512:768], in_=wT_in[:, :, 512:768])
    nc.sync.dma_start(out=woT_sb, in_=woT)

    # ---- phase 1: q/k projection + RoPE (reads PSUM directly) ----
    for s in range(4):  # 512-token slices
        tsl = slice(s * 512, (s + 1) * 512)
        accs = []
        for blk in range(4):  # q_x1 q_x2 k_x1 k_x2
            acc = p1[blk].tile([128, 512], f32, tag="p")
            for kc in range(8):
                nc.tensor.matmul(
                    acc,
                    lhsT=wT_sb[:, kc * 768 + blk * 128:kc * 768 + (blk + 1) * 128],
                    rhs=xT_sb[:, kc * T + s * 512:kc * T + (s + 1) * 512],
                    start=(kc == 0),
                    stop=(kc == 7),
                )
            accs.append(acc)
        # RoPE: rot1 = x1*cos - x2*sin ; rot2 = x2*cos + x1*sin
        for pair in range(2):  # 0 -> q, 1 -> k
            x1, x2 = accs[2 * pair], accs[2 * pair + 1]
            r1, r2 = rot[2 * pair][:, tsl], rot[2 * pair + 1][:, tsl]
            t1 = tmp.tile([128, 512], bf16, tag="t1")
            t2 = tmp.tile([128, 512], bf16, tag="t2")
            t3 = tmp.tile([128, 512], bf16, tag="t3")
            t4 = tmp.tile([128, 512], bf16, tag="t4")
            nc.vector.tensor_mul(t1, x1, cos_sb[:, tsl])
            nc.vector.tensor_mul(t2, x2, sin_sb[:, tsl])
            nc.vector.tensor_sub(r1, t1, t2)
            nc.vector.tensor_mul(t3, x2, cos_sb[:, tsl])
            nc.vector.tensor_mul(t4, x1, sin_sb[:, tsl])
            nc.vector.tensor_add(r2, t3, t4)
        # repack this token slice into head-contiguous [64, (h t)] layout
        for hl in range(HEADS_PER_CORE):
            hsl = slice(hl * T + s * 512, hl * T + (s + 1) * 512)
            for half in range(2):
                psl = slice(half * 32, (half + 1) * 32)
                rsl = slice(hl * 32, (hl + 1) * 32)
                nc.sync.dma_start(out=qhT[psl, hsl], in_=rot[half][rsl, tsl])
                nc.sync.dma_start(out=khT[psl, hsl], in_=rot[2 + half][rsl, tsl])

    # ---- phase 2: software-pipelined attention ----
    # per iteration i: v(i), bcast(i-1), attention(i)+recip(i),
    # normalize-muls(i-1), out-proj(i-1).  The tensor queue never waits on
    # the reciprocal: by the time it reaches bcast(i-1), rc4(i-1) is old.
    o4s = [None] * QT
    rc4s = [None] * QT
    rcbs = [None] * QT

    def bcast(qt):  # rcb[d, (h q)] = 1/rowsum broadcast over partitions
        # gpsimd, emitted after a head-loop's masks: it executes in the
        # mask-free window while PV3/outproj run, never blocking a mask
        rcbs[qt] = work.tile([64, 512], f32, tag="rcb", name=f"rcb{qt}")
        nc.gpsimd.partition_broadcast(rcbs[qt], rc4s[qt])

    def norm_muls(qt):  # attnT <- o4 * rcb  (both PSUM)
        ov = o4s[qt][0:64, :].rearrange("p (h q) -> p h q", h=4)
        rv = rcbs[qt].rearrange("p (h q) -> p h q", h=4)
        av = attnT.rearrange("p (b t) -> p b t", b=2)
        qsl = slice(qt * 128, (qt + 1) * 128)
        nc.vector.tensor_mul(av[0:64, :, qsl], ov[:, 0::2, :], rv[:, 0::2, :])
        nc.vector.tensor_mul(av[64:128, :, qsl], ov[:, 1::2, :], rv[:, 1::2, :])

    def outproj(qt):
        ysb = ysbp.tile([128, C], bf16, tag="ysb")
        for nh in range(2):
            acc = (pA if nh == 0 else pB).tile([128, 512], f32, tag="p")
            for kc in range(2):
                nc.tensor.matmul(
                    acc,
                    lhsT=attnT[:, kc * T + qt * 128:kc * T + (qt + 1) * 128],
                    rhs=woT_sb[:, kc * C + nh * 512:kc * C + (nh + 1) * 512],
                    start=(kc == 0),
                    stop=(kc == 1),
                )
            if nh == 0:  # vector is idle mid-iteration; scalar still on exps
                nc.vector.tensor_copy(ysb[:, nh * 512:(nh + 1) * 512], acc)
            else:
                nc.scalar.copy(ysb[:, nh * 512:(nh + 1) * 512], acc)
        nc.sync.dma_start(out=y[qt * 128:(qt + 1) * 128, :], in_=ysb)

    for qt in range(QT):
        # v tile qt in natural (token-partition) layout, strided into v4
        acc = pA.tile([128, FQ], f32, tag="p")
        for kc in range(8):
            nc.tensor.matmul(
                acc,
                lhsT=xT_sb[:, kc * T + qt * 128:kc * T + (qt + 1) * 128],
                rhs=wT_sb[:, kc * 768 + 512:kc * 768 + 768],
                start=(kc == 0),
                stop=(kc == 7),
            )
        nc.any.tensor_copy(
            v4v[:, qt, :, 0:DH],
            acc.rearrange("p (h f) -> p h f", h=4),
        )
        if qt >= 2:
            bcast(qt - 2)

        nkt = min(qt + 1, 3)  # key tiles in window
        w = 128 * nkt
        kt0 = max(qt - 2, 0)
        o4 = o4s[qt] = pO.tile([65, 512], f32, tag="p", name=f"o4_{qt}")

        def pv(hl, pT):
            # middle block (fully in-window, no mask dep) first: it can issue
            # as soon as the exp lands, before the affine_selects finish
            order = [1, 0, 2] if nkt == 3 else list(range(nkt))
            for i, a in enumerate(order):
                kt = kt0 + a
                nc.tensor.matmul(
                    o4[:, hl * 128:(hl + 1) * 128],
                    lhsT=v4[:, (kt * 4 + hl) * VG:(kt * 4 + hl + 1) * VG],
                    rhs=pT[:, a * 128:(a + 1) * 128],
                    start=(i == 0),
                    stop=(i == nkt - 1),
                )

        pend = []  # (hl, pT) whose PV issues two heads later
        for hl in range(HEADS_PER_CORE):
            # transposed scores sT[k, q] accumulate straight into one bank
            sT = pB.tile([128, 384], f32, tag="p")
            for a in range(nkt):
                nc.tensor.matmul(
                    sT[:, a * 128:(a + 1) * 128],
                    lhsT=khT[:, hl * T + (kt0 + a) * 128:hl * T + (kt0 + a + 1) * 128],
                    rhs=qhT[:, hl * T + qt * 128:hl * T + (qt + 1) * 128],
                    start=True,
                    stop=True,
                )
            pT = work.tile([128, 384], bf16, tag="pT")
            nc.scalar.activation(pT[:, :w], sT[:, :w], Exp)
            # band masks, post-exp on GpSimd: diagonal block keeps k<=q,
            # far block (qt-2) keeps k>q; middle block is fully in-window
            d0 = (nkt - 1) * 128
            nc.gpsimd.affine_select(
                pT[:, d0:d0 + 128], pT[:, d0:d0 + 128], pattern=[[1, 128]],
                compare_op=GE, fill=zero_reg, base=0, channel_multiplier=-1)
            if qt >= 2:
                nc.gpsimd.affine_select(
                    pT[:, 0:128], pT[:, 0:128], pattern=[[-1, 128]],
                    compare_op=GE, fill=zero_reg, base=-1, channel_multiplier=1)
            if len(pend) >= 3:
                pv(*pend.pop(0))
            pend.append((hl, pT))
        pv(*pend.pop(0))
        if qt >= 2:
            norm_muls(qt - 2)
            outproj(qt - 2)  # tensor work covering the last heads' exp+mask
        pv(*pend.pop(0))
        pv(*pend.pop(0))
        rs_sb = small.tile([1, 512], f32, tag="rs")
        nc.scalar.activation(rs_sb, o4[64:65, :], Copy)
        rc4s[qt] = small.tile([1, 512], f32, tag="rc4", name=f"rc4_{qt}")
        nc.vector.reciprocal_approx_fast(rc4s[qt], rs_sb)
    for qt in (QT - 2, QT - 1):
        bcast(qt)
        norm_muls(qt)
        outproj(qt)


def _build_program():
    import concourse.tile as tile
    from concourse import bacc, mybir

    bf16 = mybir.dt.bfloat16

    nc = bacc.Bacc("TRN2", target_bir_lowering=False, debug=False,
                   num_devices=N_CORES)
    aps = {
        "xT": nc.dram_tensor("xT", [128, 8 * T], bf16, kind="ExternalInput").ap(),
        "wT": nc.dram_tensor("wT", [128, 8 * 768], bf16, kind="ExternalInput").ap(),
        "woT": nc.dram_tensor("woT", [128, 2 * C], bf16, kind="ExternalInput").ap(),
        "cos4": nc.dram_tensor("cos4", [128, T], bf16, kind="ExternalInput").ap(),
        "sin4": nc.dram_tensor("sin4", [128, T], bf16, kind="ExternalInput").ap(),
        "y": nc.dram_tensor("y", [T, C], bf16, kind="ExternalOutput").ap(),
    }
    from contextlib import ExitStack

    with tile.TileContext(nc) as tc, ExitStack() as ctx:
        _emit(nc, tc, aps, ctx)
    nc.compile()
    return nc


def _get_program():
    global _PROGRAM
    if _PROGRAM is None:
        _PROGRAM = _build_program()
    return _PROGRAM


def _host_inputs(x, w_qkv, w_out):
    import ml_dtypes

    bf16 = ml_dtypes.bfloat16
    x = np.asarray(x, np.float32)
    w_qkv = np.asarray(w_qkv, np.float32)
    w_out = np.asarray(w_out, np.float32)

    wq, wk, wv = w_qkv[0:C], w_qkv[C:2 * C], w_qkv[2 * C:3 * C]
    scale = 1.0 / math.sqrt(DH)

    # RoPE tables (transposed, tiled over the 4 heads of a block)
    inv_freq = 1.0 / (10000.0 ** (np.arange(0, DH, 2, dtype=np.float32) / DH))
    freqs = np.outer(np.arange(T, dtype=np.float32), inv_freq)  # [T, 32]
    cos4 = np.ascontiguousarray(np.tile(np.cos(freqs).T, (4, 1))).astype(bf16)
    sin4 = np.ascontiguousarray(np.tile(np.sin(freqs).T, (4, 1))).astype(bf16)

    def ilv(m):  # [(kc*128), F] -> [128, kc*F] kc-major interleave
        kc = m.shape[0] // 128
        return np.ascontiguousarray(
            m.reshape(kc, 128, -1).transpose(1, 0, 2).reshape(128, -1))

    xT = [ilv(x[b].T).astype(bf16) for b in range(B)]

    in_maps = []
    for core in range(N_CORES):
        b, g = divmod(core, 4)
        hs = range(4 * g, 4 * g + 4)
        rows = []
        for half in range(2):  # q_x1, q_x2
            rows.append(np.concatenate(
                [wq[h * DH + 32 * half:h * DH + 32 * half + 32] for h in hs]) * scale)
        for half in range(2):  # k_x1, k_x2
            rows.append(np.concatenate(
                [wk[h * DH + 32 * half:h * DH + 32 * half + 32] for h in hs]))
        rows.append(wv[g * FQ:(g + 1) * FQ])
        wmat = np.concatenate(rows)  # [768, C]
        wT = ilv(wmat.T).astype(bf16)
        woT = ilv(w_out[:, g * FQ:(g + 1) * FQ].T).astype(bf16)
        in_maps.append({
            "xT": xT[b], "wT": wT, "woT": woT, "cos4": cos4, "sin4": sin4,
        })
    return in_maps


def kernel(x, w_qkv, w_out, _trace=False):
    from concourse import bass_utils

    nc = _get_program()
    in_maps = _host_inputs(x, w_qkv, w_out)
    res = bass_utils.run_bass_kernel_spmd(
        nc, in_maps, core_ids=list(range(N_CORES)), trace=_trace,
    )
    parts = [res.results[core]["y"].astype(np.float32) for core in range(N_CORES)]
    out = np.stack([
        parts[0] + parts[1] + parts[2] + parts[3],
        parts[4] + parts[5] + parts[6] + parts[7],
    ])
    if _trace:
        return out, res
    return out
